# revision 1
# baseline (speedup 1.0000x reference)
# Trainium2 Bass kernel for nn_AxialAttention (8 NeuronCores, head/W-parallel).
#
# Sharding: the W axis (axis=2, the vmapped axis) is split into 8 contiguous
# slices of 32 columns, one per core. Every part of the computation (the four
# 1x1-conv GEMMs, the per-(head, w) axial attention, the embedding terms) is
# independent across w, so there are no collectives; the small weight matrices
# and embedding tables are replicated to every core.
#
# Per-core math for one w column (all heads):
#   qsT[x, (h c)] = query[:, :, w].T @ (Wq.T / 16)    (scale folded into Wq)
#   khT[x, (h c)] = key_[:, :, w].T @ Wk.T
#   vh [(h c), x] = Wv @ value[:, :, w]
#   logits_h[C, c] = khT_h.T @ qsT_h + q_emb.T @ qsT_h + k_emb.T @ khT_h
#   E = exp(logits)             (max-subtraction unnecessary: |logits| < ~2)
#   U_h = E_h.T @ [vh_h + ve | 1]          (ones column gives the softmax
#   attn_h = U_h[:, :256] / U_h[:, 256]     denominator for free)
#   out[:, :, w] = Wo @ attn
#
# Heads are packed even/odd into the two 64-partition halves so the per-head
# 64x64 logits matmuls and the 64-row attention matmuls run as concurrent
# PE row/column tiles (tile_position diagonal packing). All matmuls run in
# bf16 with fp32 PSUM accumulation (measured 3.3e-3 absmax-relative error);
# large PSUM->SBUF evacuations are split across the Scalar and Vector engines
# to halve PSUM-slot release latency.

import numpy as np

H = 8          # heads
QK = 64        # per-head qk/vo channels
C = 512        # io channels
X = 256        # spatial H (attention contraction axis)
W = 256        # spatial W (vmapped axis, sharded)
N_CORES = 8
WC = W // N_CORES   # w columns per core
PAIRS = WC // 2

_CACHE = {}


def _build_program():
    import concourse.mybir as mybir
    import concourse.tile as tile
    from concourse import bacc

    f32 = mybir.dt.float32
    bf16 = mybir.dt.bfloat16
    AF = mybir.ActivationFunctionType

    nc = bacc.Bacc("TRN2", target_bir_lowering=False, debug=False,
                   num_devices=N_CORES)

    qin = nc.dram_tensor("qin", [PAIRS, C, 2, X], bf16, kind="ExternalInput").ap()
    kin = nc.dram_tensor("kin", [PAIRS, C, 2, X], bf16, kind="ExternalInput").ap()
    vin = nc.dram_tensor("vin", [PAIRS, C, 2, X], bf16, kind="ExternalInput").ap()
    wqt = nc.dram_tensor("wqt", [C, C], bf16, kind="ExternalInput").ap()
    wkt = nc.dram_tensor("wkt", [C, C], bf16, kind="ExternalInput").ap()
    wvt = nc.dram_tensor("wvt", [C, C], bf16, kind="ExternalInput").ap()
    wot = nc.dram_tensor("wot", [C, C], bf16, kind="ExternalInput").ap()
    qe8 = nc.dram_tensor("qe8", [X, H * QK], bf16, kind="ExternalInput").ap()
    ke2 = nc.dram_tensor("ke2", [X, 2 * QK], bf16, kind="ExternalInput").ap()
    vet = nc.dram_tensor("vet", [QK, X], f32, kind="ExternalInput").ap()
    oned = nc.dram_tensor("oned", [128, 4], bf16, kind="ExternalInput").ap()
    out = nc.dram_tensor("out", [C, WC, X], f32, kind="ExternalOutput").ap()

    KT = C // 128   # 4 contraction tiles of the channel dim
    XT = X // 128   # 2 tiles of the spatial-x dim

    with tile.TileContext(nc) as tc:
        with (
            tc.tile_pool(name="consts", bufs=1) as consts,
            tc.tile_pool(name="inp", bufs=3) as inp,
            tc.tile_pool(name="qkt", bufs=2) as qkt,
            tc.tile_pool(name="mid", bufs=2) as mid,
            tc.tile_pool(name="small", bufs=8) as small,
            tc.tile_pool(name="psA", bufs=3, space="PSUM") as psA,
            tc.tile_pool(name="psVL", bufs=2, space="PSUM") as psVL,
            tc.tile_pool(name="psU", bufs=3, space="PSUM") as psU,
        ):
            def load_inputs(pair):
                q_t = inp.tile([128, KT, 2, X], bf16, tag="q_t")
                nc.sync.dma_start(
                    q_t[:], qin[pair].rearrange("(kt p) w x -> p kt (w x)", p=128))
                k_t = inp.tile([128, KT, 2, X], bf16, tag="k_t")
                nc.sync.dma_start(
                    k_t[:], kin[pair].rearrange("(kt p) w x -> p kt (w x)", p=128))
                v_t = inp.tile([128, KT, 2, X], bf16, tag="v_t")
                nc.sync.dma_start(
                    v_t[:], vin[pair].rearrange("(kt p) w x -> p kt (w x)", p=128))
                return q_t, k_t, v_t

            # pair-0 inputs first so the PE can start ASAP; q is split per
            # k-tile so the first matmul only waits for one chunk. Constants
            # go on the ACT HWDGE ring so the two DMA streams run in parallel.
            q0 = inp.tile([128, KT, 2, X], bf16, tag="q_t")
            for kt in range(KT):
                nc.sync.dma_start(
                    q0[:, kt, :, :],
                    qin[0, kt * 128:(kt + 1) * 128].rearrange("p w x -> p (w x)"))
            k0 = inp.tile([128, KT, 2, X], bf16, tag="k_t")
            nc.sync.dma_start(
                k0[:], kin[0].rearrange("(kt p) w x -> p kt (w x)", p=128))
            v0 = inp.tile([128, KT, 2, X], bf16, tag="v_t")
            nc.sync.dma_start(
                v0[:], vin[0].rearrange("(kt p) w x -> p kt (w x)", p=128))
            prefetched = (q0, k0, v0)

            wq_sb = consts.tile([128, KT, C], bf16)
            nc.scalar.dma_start(wq_sb[:], wqt.rearrange("(kt p) o -> p kt o", p=128))
            wk_sb = consts.tile([128, KT, C], bf16)
            nc.scalar.dma_start(wk_sb[:], wkt.rearrange("(kt p) o -> p kt o", p=128))
            wv_sb = consts.tile([128, KT, C], bf16)
            nc.scalar.dma_start(wv_sb[:], wvt.rearrange("(kt p) o -> p kt o", p=128))
            wo_sb = consts.tile([128, KT, C], bf16)
            nc.scalar.dma_start(wo_sb[:], wot.rearrange("(kt p) o -> p kt o", p=128))
            qe8_sb = consts.tile([128, XT, H * QK], bf16)
            nc.scalar.dma_start(qe8_sb[:], qe8.rearrange("(xt p) m -> p xt m", p=128))
            ke_sb = consts.tile([128, XT, 2 * QK], bf16)
            nc.scalar.dma_start(ke_sb[:], ke2.rearrange("(xt p) m -> p xt m", p=128))
            ve_sb = consts.tile([128, X], f32)
            nc.scalar.dma_start(ve_sb[0:QK, :], vet[:])
            nc.scalar.dma_start(ve_sb[QK:128, :], vet[:])
            ones_sb = consts.tile([128, 2, 2], bf16)
            nc.scalar.dma_start(ones_sb[:], oned.rearrange("p (a b) -> p a b", a=2))

            for pair in range(PAIRS):
                w0 = pair * 2
                q_t, k_t, v_t = prefetched if pair == 0 else load_inputs(pair)

                # --- q/k projections, transposed layout: qsT/khT [x, (h c)] ---
                qsT = qkt.tile([128, 2, XT, C], bf16)   # [x_p, w, xt, o]
                khT = qkt.tile([128, 2, XT, C], bf16)
                khq = qkt.tile([128, 2, XT, C], bf16)   # khT + q_emb (folds t2)
                for wi in range(2):
                    for xt in range(XT):
                        pq = psA.tile([128, C], f32, tag="mm")
                        for kt in range(KT):
                            nc.tensor.matmul(
                                pq[:],
                                q_t[:, kt, wi, xt * 128:(xt + 1) * 128],
                                wq_sb[:, kt, :],
                                start=(kt == 0), stop=(kt == KT - 1))
                        nc.scalar.activation(qsT[:, wi, xt, 0:256], pq[:, 0:256],
                                             AF.Copy)
                        nc.vector.tensor_copy(qsT[:, wi, xt, 256:512],
                                              pq[:, 256:512])
                        pk = psA.tile([128, C], f32, tag="mm")
                        for kt in range(KT):
                            nc.tensor.matmul(
                                pk[:],
                                k_t[:, kt, wi, xt * 128:(xt + 1) * 128],
                                wk_sb[:, kt, :],
                                start=(kt == 0), stop=(kt == KT - 1))
                        nc.vector.tensor_copy(khT[:, wi, xt, 0:256], pk[:, 0:256])
                        nc.scalar.activation(khT[:, wi, xt, 256:512],
                                             pk[:, 256:512], AF.Copy)
                        nc.gpsimd.tensor_add(khq[:, wi, xt, :],
                                             khT[:, wi, xt, :], qe8_sb[:, xt, :])

                # --- v projection + ve add + ones column ---
                vplus = mid.tile([128, KT, 2, X + 2], bf16)  # [c2_p, head-pair, w, x+2]
                for ot in range(KT):
                    pv = psVL.tile([128, 2, X], f32, tag="vl")
                    for kt in range(KT):
                        nc.tensor.matmul(
                            pv[:],
                            wv_sb[:, kt, ot * 128:(ot + 1) * 128],
                            v_t[:, kt, :, :],
                            start=(kt == 0), stop=(kt == KT - 1))
                    for wi in range(2):
                        nc.vector.tensor_add(
                            vplus[:, ot, wi, 0:X], pv[:, wi, :], ve_sb[:])
                    nc.vector.tensor_copy(vplus[:, ot, :, X:X + 2], ones_sb[:])

                # --- per-w attention ---
                attn = mid.tile([128, KT, 2, X], bf16)  # [(h c)_p, kt, w, x]
                for wi in range(2):
                    pl = psVL.tile([128, C], f32, tag="vl")
                    # k_emb term, all heads at once (dup'd table)
                    nc.tensor.matmul(pl[:], ke_sb[:, 0, :], khT[:, wi, 0, :],
                                     start=True, stop=False)
                    nc.tensor.matmul(pl[:], ke_sb[:, 1, :], khT[:, wi, 1, :],
                                     start=False, stop=False)
                    # per-head (kh + qe)^T @ qs term (folds the q_emb term)
                    for h in range(H):
                        half = (h % 2) * QK
                        cb = h * QK
                        for xt in range(XT):
                            nc.tensor.matmul(
                                pl[half:half + QK, cb:cb + QK],
                                khq[:, wi, xt, cb:cb + QK],
                                qsT[:, wi, xt, cb:cb + QK],
                                start=False, stop=(h == H - 1 and xt == XT - 1),
                                tile_position=(0, half))
                    e_t = mid.tile([128, C], bf16, tag="e")
                    nc.scalar.activation(e_t[:], pl[:], AF.Exp)

                    for t in range(KT):          # head pairs (2t, 2t+1)
                        pu = psU.tile([128, X + 2], f32, tag="pu")
                        for j in range(2):       # j=0 even head, j=1 odd head
                            h = 2 * t + j
                            half = j * QK
                            nc.tensor.matmul(
                                pu[half:half + QK, :],
                                e_t[half:half + QK, h * QK:(h + 1) * QK],
                                vplus[half:half + QK, t, wi, :],
                                start=True, stop=True,
                                tile_position=(half, half))
                        recip = small.tile([128, 1], f32, tag="recip")
                        nc.vector.reciprocal(recip[:], pu[:, X:X + 1])
                        if t % 2 == 0:
                            nc.scalar.activation(
                                attn[:, t, wi, :],
                                pu[:, 0:X], AF.Copy, scale=recip[:])
                        else:
                            nc.vector.tensor_scalar_mul(
                                attn[:, t, wi, :], pu[:, 0:X], recip[:])

                # --- output projection ---
                for ot in range(KT):
                    po = psVL.tile([128, 2, X], f32, tag="vl")
                    for kt in range(KT):
                        nc.tensor.matmul(
                            po[:],
                            wo_sb[:, kt, ot * 128:(ot + 1) * 128],
                            attn[:, kt, :, :],
                            start=(kt == 0), stop=(kt == KT - 1))
                    ob = mid.tile([128, 2, X], f32, tag="ob")
                    nc.scalar.activation(ob[:, 0, :], po[:, 0, :], AF.Copy)
                    nc.vector.tensor_copy(ob[:, 1, :], po[:, 1, :])
                    nc.sync.dma_start(
                        out[ot * 128:(ot + 1) * 128, w0:w0 + 2, :], ob[:])

    nc.compile()
    return nc


def _get_program():
    if "nc" not in _CACHE:
        _CACHE["nc"] = _build_program()
    return _CACHE["nc"]


def _make_in_maps(query, key_, value, Wq, Wk, Wv, Wo, q_emb, k_emb, v_emb):
    import ml_dtypes
    bf16 = ml_dtypes.bfloat16
    scale = np.float32(1.0 / np.sqrt(X))
    wqt = np.ascontiguousarray((Wq.T * scale).astype(bf16))
    wkt = np.ascontiguousarray(Wk.T.astype(bf16))
    wvt = np.ascontiguousarray(Wv.T.astype(bf16))
    wot = np.ascontiguousarray(Wo.T.astype(bf16))
    qe8 = np.ascontiguousarray(np.tile(q_emb, (1, H)).astype(bf16))
    ke2 = np.ascontiguousarray(np.concatenate([k_emb, k_emb], axis=1).astype(bf16))
    vet = np.ascontiguousarray(v_emb.T)
    def shard(a, ws):
        # (C, X, WC) -> [pair, i, w, x] contiguous, bf16
        return np.ascontiguousarray(
            a[:, :, ws].reshape(C, X, PAIRS, 2).transpose(2, 0, 3, 1).astype(bf16))

    in_maps = []
    for c in range(N_CORES):
        ws = slice(c * WC, (c + 1) * WC)
        in_maps.append({
            "qin": shard(query, ws),
            "kin": shard(key_, ws),
            "vin": shard(value, ws),
            "wqt": wqt, "wkt": wkt, "wvt": wvt, "wot": wot,
            "qe8": qe8, "ke2": ke2, "vet": vet,
            "oned": np.ones((128, 4), bf16),
        })
    return in_maps


def _run(in_maps, trace=False):
    from concourse.bass_utils import run_bass_kernel_spmd
    nc = _get_program()
    return run_bass_kernel_spmd(nc, in_maps, list(range(N_CORES)), trace=trace)


def kernel(query, key_, value, Wq, Wk, Wv, Wo, q_emb, k_emb, v_emb):
    args = (query, key_, value, Wq, Wk, Wv, Wo, q_emb, k_emb, v_emb)
    in_maps = _make_in_maps(*[np.ascontiguousarray(a, np.float32) for a in args])
    res = _run(in_maps, trace=False)
    out = np.empty((C, X, W), np.float32)
    for c in range(N_CORES):
        out[:, :, c * WC:(c + 1) * WC] = res.results[c]["out"].transpose(0, 2, 1)
    return out



# revision 18
# speedup vs baseline: 1.0211x; 1.0211x over previous
# Trainium2 Bass kernel for nn_AxialAttention (8 NeuronCores, W-parallel).
#
# Sharding: the W axis (axis=2, the vmapped axis) is split into 8 contiguous
# slices of 32 columns, one per core. Every part of the computation (the four
# 1x1-conv GEMMs, the per-(head, w) axial attention, the embedding terms) is
# independent across w, so there are no collectives; the small weight matrices
# and embedding tables are replicated to every core.
#
# Per-core math for one w column (all heads):
#   qsT[x, (h c)] = query[:, :, w].T @ (Wq.T / 16)    (scale folded into Wq)
#   khT[x, (h c)] = key_[:, :, w].T @ Wk.T
#   vh [(h c), x] = Wv @ value[:, :, w]
#   logits_h[C, c] = khT_h.T @ qsT_h + q_emb.T @ qsT_h + k_emb.T @ khT_h
#   E = exp(logits)             (max-subtraction unnecessary: |logits| < ~2)
#   U_h = E_h.T @ [vh_h + ve | 1]          (ones column gives the softmax
#   attn_h = U_h[:, :256] / U_h[:, 256]     denominator for free)
#   out[:, :, w] = Wo @ attn
#
# Precision strategy (validated against the fp32 reference, rel-err ~1.1e-2
# vs the 2e-2 gate): the q/k projections and all three logits matmul terms
# run in fp8 E4M3 with DoubleRow perf mode (2x PE throughput); the softmax
# smooths the fp8 error.  The v/o projections stay bf16 (fp8 there is a
# direct additive error path and fails the gate).  Power-of-2 pre-scales
# keep fp8 operands in range: wq8/wk8 = W.T*32, qsT/khT evacuate to fp8 at
# 512x/32x their true scale, q_emb x32, k_emb x512; the 16384x logit scale
# is removed for free inside the Exp activation (scale=1/16384).
#
# PSUM discipline: each 2KB PSUM bank holds exactly one accumulation group;
# the first matmul of a group has start=True (hardware zeroes the whole
# bank), every other matmul accumulates (start=False), including ones that
# touch bytes no earlier instruction wrote (they add onto the zeroed bank).
#
# Heads are packed even/odd into the two 64-partition halves so the per-head
# 64x64 logits matmuls and the 64-row attention matmuls run as concurrent
# PE row/column tiles (tile_position diagonal packing).  Large PSUM->SBUF
# evacuations are spread across the Scalar/Vector/GpSimd engines.

import numpy as np

H = 8          # heads
QK = 64        # per-head qk/vo channels
C = 512        # io channels
X = 256        # spatial H (attention contraction axis)
W = 256        # spatial W (vmapped axis, sharded)
N_CORES = 8
WC = W // N_CORES   # w columns per core
PAIRS = WC // 2

_CACHE = {}


def _build_program():
    import concourse.mybir as mybir
    import concourse.tile as tile
    from concourse import bacc

    f32 = mybir.dt.float32
    bf16 = mybir.dt.bfloat16
    f8 = mybir.dt.float8e4
    AF = mybir.ActivationFunctionType
    DR = mybir.MatmulPerfMode.DoubleRow

    nc = bacc.Bacc("TRN2", target_bir_lowering=False, debug=False,
                   num_devices=N_CORES)

    qin = nc.dram_tensor("qin", [PAIRS, C, 2, X], f8, kind="ExternalInput").ap()
    kin = nc.dram_tensor("kin", [PAIRS, C, 2, X], f8, kind="ExternalInput").ap()
    vin = nc.dram_tensor("vin", [PAIRS, C, 2, X], bf16, kind="ExternalInput").ap()
    wq8 = nc.dram_tensor("wq8", [C, C], f8, kind="ExternalInput").ap()
    wk8 = nc.dram_tensor("wk8", [C, C], f8, kind="ExternalInput").ap()
    wvt = nc.dram_tensor("wvt", [C, C], bf16, kind="ExternalInput").ap()
    wot = nc.dram_tensor("wot", [C, C], bf16, kind="ExternalInput").ap()
    qe8 = nc.dram_tensor("qe8", [X, 2 * QK], f8, kind="ExternalInput").ap()
    ke8 = nc.dram_tensor("ke8", [X, 2 * QK], f8, kind="ExternalInput").ap()
    vet = nc.dram_tensor("vet", [QK, X], f32, kind="ExternalInput").ap()
    out = nc.dram_tensor("out", [C, WC, X], f32, kind="ExternalOutput").ap()

    KT = C // 128   # 4 contraction tiles of the channel dim
    XT = X // 128   # 2 tiles of the spatial-x dim
    ELS = 1.0 / 16384.0   # logit descale folded into Exp

    with tile.TileContext(nc) as tc:
        with (
            tc.tile_pool(name="consts", bufs=1) as consts,
            tc.tile_pool(name="inp", bufs=3) as inp,
            tc.tile_pool(name="qkt", bufs=2) as qkt,
            tc.tile_pool(name="mid", bufs=2) as mid,
            tc.tile_pool(name="small", bufs=8) as small,
            tc.tile_pool(name="psA", bufs=3, space="PSUM") as psA,
            tc.tile_pool(name="psVL", bufs=2, space="PSUM") as psVL,
            tc.tile_pool(name="psU", bufs=3, space="PSUM") as psU,
        ):
            def load_inputs(pair):
                # fp8 q/k: channel = (kp*2 + ki)*128 + p  ->  [p, kp, ki, w, x]
                q_t = inp.tile([128, 2, 2, 2, X], f8, tag="q_t")
                nc.sync.dma_start(
                    q_t[:], qin[pair].rearrange(
                        "(kp ki p) w x -> p kp ki (w x)", p=128, ki=2))
                k_t = inp.tile([128, 2, 2, 2, X], f8, tag="k_t")
                nc.sync.dma_start(
                    k_t[:], kin[pair].rearrange(
                        "(kp ki p) w x -> p kp ki (w x)", p=128, ki=2))
                v_t = inp.tile([128, KT, 2, X], bf16, tag="v_t")
                nc.sync.dma_start(
                    v_t[:], vin[pair].rearrange("(kt p) w x -> p kt (w x)", p=128))
                return q_t, k_t, v_t

            # pair-0 inputs first so the PE can start ASAP; q is split per
            # k-tile so the first matmul only waits for one chunk. Constants
            # go on the ACT HWDGE ring so the two DMA streams run in parallel.
            q0 = inp.tile([128, 2, 2, 2, X], f8, tag="q_t")
            for kp in range(2):
                nc.sync.dma_start(
                    q0[:, kp, :, :, :],
                    qin[0, kp * 256:(kp + 1) * 256].rearrange(
                        "(ki p) w x -> p ki w x", p=128))
            k0 = inp.tile([128, 2, 2, 2, X], f8, tag="k_t")
            nc.sync.dma_start(
                k0[:], kin[0].rearrange(
                    "(kp ki p) w x -> p kp ki (w x)", p=128, ki=2))
            v0 = inp.tile([128, KT, 2, X], bf16, tag="v_t")
            nc.sync.dma_start(
                v0[:], vin[0].rearrange("(kt p) w x -> p kt (w x)", p=128))
            prefetched = (q0, k0, v0)

            wq_sb = consts.tile([128, 2, 2, C], f8)
            nc.scalar.dma_start(wq_sb[:], wq8.rearrange(
                "(kp ki p) o -> p kp ki o", p=128, ki=2))
            wk_sb = consts.tile([128, 2, 2, C], f8)
            nc.scalar.dma_start(wk_sb[:], wk8.rearrange(
                "(kp ki p) o -> p kp ki o", p=128, ki=2))
            wv_sb = consts.tile([128, KT, C], bf16)
            nc.scalar.dma_start(wv_sb[:], wvt.rearrange("(kt p) o -> p kt o", p=128))
            wo_sb = consts.tile([128, KT, C], bf16)
            nc.scalar.dma_start(wo_sb[:], wot.rearrange("(kt p) o -> p kt o", p=128))
            qe_sb = consts.tile([128, XT, 2 * QK], f8)
            nc.scalar.dma_start(qe_sb[:], qe8.rearrange("(xt p) m -> p xt m", p=128))
            ke_sb = consts.tile([128, XT, 2 * QK], f8)
            nc.scalar.dma_start(ke_sb[:], ke8.rearrange("(xt p) m -> p xt m", p=128))
            ve_sb = consts.tile([128, 1, X], f32)
            nc.scalar.dma_start(ve_sb[0:QK, 0, :], vet[:])
            nc.scalar.dma_start(ve_sb[QK:128, 0, :], vet[:])


            for pair in range(PAIRS):
                w0 = pair * 2
                q_t, k_t, v_t = prefetched if pair == 0 else load_inputs(pair)

                # --- q/k projections (fp8 DoubleRow), transposed layout:
                #     qsT/khT [x, (h c)] evacuated straight to fp8 ---
                qsT = qkt.tile([128, 2, XT, C], f8)   # [x_p, w, xt, o]
                khT = qkt.tile([128, 2, XT, C], f8)
                for wi in range(2):
                    for xt in range(XT):
                        xs = slice(xt * 128, (xt + 1) * 128)
                        pq = psA.tile([128, C], f32, tag="mm")
                        first = True
                        for kp in range(2):
                            for nh in range(2):
                                ns = slice(nh * 256, (nh + 1) * 256)
                                nc.tensor.matmul(
                                    pq[:, ns],
                                    q_t[:, kp, :, wi, xs],
                                    wq_sb[:, kp, :, ns],
                                    start=first,
                                    stop=(kp == 1 and nh == 1),
                                    perf_mode=DR)
                                first = False
                        nc.scalar.activation(qsT[:, wi, xt, :], pq[:], AF.Copy)
                        pk = psA.tile([128, C], f32, tag="mm")
                        first = True
                        for kp in range(2):
                            for nh in range(2):
                                ns = slice(nh * 256, (nh + 1) * 256)
                                nc.tensor.matmul(
                                    pk[:, ns],
                                    k_t[:, kp, :, wi, xs],
                                    wk_sb[:, kp, :, ns],
                                    start=first,
                                    stop=(kp == 1 and nh == 1),
                                    perf_mode=DR)
                                first = False
                        nc.vector.tensor_copy(khT[:, wi, xt, :], pk[:])

                # --- v projection + ve add + ones column (bf16) ---
                vplus = mid.tile([128, KT, 2, X + 2], bf16)  # [c2_p, hp, w, x+2]
                for ot in range(KT):
                    pv = psVL.tile([128, 2, X], f32, tag="vl")
                    for kt in range(KT):
                        nc.tensor.matmul(
                            pv[:],
                            wv_sb[:, kt, ot * 128:(ot + 1) * 128],
                            v_t[:, kt, :, :],
                            start=(kt == 0), stop=(kt == KT - 1))
                    nc.vector.tensor_add(
                        vplus[:, ot, :, 0:X], pv[:],
                        ve_sb[:].broadcast_to([128, 2, X]))
                nc.gpsimd.memset(vplus[:, :, :, X:X + 2], 1.0)

                # --- per-w attention: logits+exp for both w first (so the
                #     Exp latency of w0 hides behind w1's logits), then U ---
                attn = mid.tile([128, KT, 2, X], bf16)  # [(h c)_p, kt, w, x]
                e_ts = []
                for wi in range(2):
                    pl = psVL.tile([128, C], f32, tag="vl")
                    # k_emb and q_emb terms, all heads at once (dup'd tables),
                    # then the per-head kh^T @ qs terms; one accumulation
                    # group for the whole bank.
                    for nh in range(2):
                        ns = slice(nh * 256, (nh + 1) * 256)
                        nc.tensor.matmul(pl[:, ns], ke_sb[:], khT[:, wi, :, ns],
                                         start=(nh == 0), stop=False,
                                         perf_mode=DR)
                    for nh in range(2):
                        ns = slice(nh * 256, (nh + 1) * 256)
                        nc.tensor.matmul(pl[:, ns], qe_sb[:], qsT[:, wi, :, ns],
                                         start=False, stop=False,
                                         perf_mode=DR)
                    for h in range(H):
                        half = (h % 2) * QK
                        cb = h * QK
                        cs = slice(cb, cb + QK)
                        if half == 0:
                            # DoubleRow only supports PE column base 0
                            nc.tensor.matmul(
                                pl[0:QK, cs],
                                khT[:, wi, :, cs],
                                qsT[:, wi, :, cs],
                                start=False, stop=(h == H - 1),
                                perf_mode=DR,
                                tile_position=(0, 0))
                        else:
                            for xt in range(XT):
                                nc.tensor.matmul(
                                    pl[half:half + QK, cs],
                                    khT[:, wi, xt, cs],
                                    qsT[:, wi, xt, cs],
                                    start=False,
                                    stop=(h == H - 1 and xt == XT - 1),
                                    tile_position=(0, half))
                    e_t = mid.tile([128, C], bf16, tag="e")
                    nc.scalar.activation(e_t[:], pl[:], AF.Exp, scale=ELS)
                    e_ts.append(e_t)

                for wi in range(2):
                    e_t = e_ts[wi]
                    for t in range(KT):          # head pairs (2t, 2t+1)
                        pu = psU.tile([128, X + 2], f32, tag="pu")
                        for j in range(2):       # j=0 even head, j=1 odd head
                            h = 2 * t + j
                            half = j * QK
                            nc.tensor.matmul(
                                pu[half:half + QK, :],
                                e_t[half:half + QK, h * QK:(h + 1) * QK],
                                vplus[half:half + QK, t, wi, :],
                                start=True, stop=True,
                                tile_position=(half, half))
                        recip = small.tile([128, 1], f32, tag="recip")
                        nc.vector.reciprocal(recip[:], pu[:, X:X + 1])
                        nc.scalar.activation(
                            attn[:, t, wi, :],
                            pu[:, 0:X], AF.Copy, scale=recip[:])

                # --- output projection (bf16) ---
                for ot in range(KT):
                    po = psVL.tile([128, 2, X], f32, tag="vl")
                    for kt in range(KT):
                        nc.tensor.matmul(
                            po[:],
                            wo_sb[:, kt, ot * 128:(ot + 1) * 128],
                            attn[:, kt, :, :],
                            start=(kt == 0), stop=(kt == KT - 1))
                    ob = mid.tile([128, 2, X], f32, tag="ob")
                    nc.vector.tensor_copy(ob[:], po[:])
                    nc.sync.dma_start(
                        out[ot * 128:(ot + 1) * 128, w0:w0 + 2, :], ob[:])

    nc.compile()
    return nc


def _get_program():
    if "nc" not in _CACHE:
        _CACHE["nc"] = _build_program()
    return _CACHE["nc"]


def _make_in_maps(query, key_, value, Wq, Wk, Wv, Wo, q_emb, k_emb, v_emb):
    import ml_dtypes
    bf16 = ml_dtypes.bfloat16
    f8 = ml_dtypes.float8_e4m3
    # fp8 pre-scales: wq folds the 1/sqrt(256) softmax scale (/16*512 = *32),
    # so qsT evacuates at 512x true scale and khT at 32x; the logits come out
    # 512*32 = 16384x and the Exp activation descales them for free.
    wq8v = np.ascontiguousarray((Wq.T * np.float32(32.0)).astype(f8))
    wk8v = np.ascontiguousarray((Wk.T * np.float32(32.0)).astype(f8))
    wvt = np.ascontiguousarray(Wv.T.astype(bf16))
    wot = np.ascontiguousarray(Wo.T.astype(bf16))
    qe8v = np.ascontiguousarray(
        np.concatenate([q_emb, q_emb], axis=1).astype(np.float32) * 32.0
    ).astype(f8)
    ke8v = np.ascontiguousarray(
        np.concatenate([k_emb, k_emb], axis=1).astype(np.float32) * 512.0
    ).astype(f8)
    vet = np.ascontiguousarray(v_emb.T)

    def shard8(a, ws):
        return np.ascontiguousarray(
            a[:, :, ws].reshape(C, X, PAIRS, 2).transpose(2, 0, 3, 1).astype(f8))

    def shardb(a, ws):
        return np.ascontiguousarray(
            a[:, :, ws].reshape(C, X, PAIRS, 2).transpose(2, 0, 3, 1).astype(bf16))

    in_maps = []
    for c in range(N_CORES):
        ws = slice(c * WC, (c + 1) * WC)
        in_maps.append({
            "qin": shard8(query, ws),
            "kin": shard8(key_, ws),
            "vin": shardb(value, ws),
            "wq8": wq8v, "wk8": wk8v, "wvt": wvt, "wot": wot,
            "qe8": qe8v, "ke8": ke8v, "vet": vet,
        })
    return in_maps


def _run(in_maps, trace=False):
    from concourse.bass_utils import run_bass_kernel_spmd
    nc = _get_program()
    return run_bass_kernel_spmd(nc, in_maps, list(range(N_CORES)), trace=trace)


def kernel(query, key_, value, Wq, Wk, Wv, Wo, q_emb, k_emb, v_emb):
    args = (query, key_, value, Wq, Wk, Wv, Wo, q_emb, k_emb, v_emb)
    in_maps = _make_in_maps(*[np.ascontiguousarray(a, np.float32) for a in args])
    res = _run(in_maps, trace=False)
    out = np.empty((C, X, W), np.float32)
    for c in range(N_CORES):
        out[:, :, c * WC:(c + 1) * WC] = res.results[c]["out"].transpose(0, 2, 1)
    return out


# revision 22
# speedup vs baseline: 1.1017x; 1.0789x over previous
# Trainium2 Bass kernel for nn_AxialAttention (8 NeuronCores, W-parallel).
#
# Sharding: the W axis (axis=2, the vmapped axis) is split into 8 contiguous
# slices of 32 columns, one per core. Every part of the computation (the four
# 1x1-conv GEMMs, the per-(head, w) axial attention, the embedding terms) is
# independent across w, so there are no collectives; the small weight matrices
# and embedding tables are replicated to every core.
#
# Per-core math for one w column (all heads):
#   qsT[x, (h c)] = query[:, :, w].T @ (Wq.T / 16)    (scale folded into Wq)
#   khT[x, (h c)] = key_[:, :, w].T @ Wk.T
#   vh [(h c), x] = Wv @ value[:, :, w]
#   logits_h[C, c] = khT_h.T @ qsT_h + q_emb.T @ qsT_h + k_emb.T @ khT_h
#   E = exp(logits)             (max-subtraction unnecessary: |logits| < ~2)
#   U_h = E_h.T @ [vh_h + ve | 1]          (ones column gives the softmax
#   attn_h = U_h[:, :256] / U_h[:, 256]     denominator for free)
#   out[:, :, w] = Wo @ attn
#
# Precision strategy (validated against the fp32 reference, rel-err ~1.1e-2
# vs the 2e-2 gate): the q/k projections and all three logits matmul terms
# run in fp8 E4M3 with DoubleRow perf mode (2x PE throughput); the softmax
# smooths the fp8 error.  The v/o projections stay bf16 (fp8 there is a
# direct additive error path and fails the gate).  Power-of-2 pre-scales
# keep fp8 operands in range: wq8/wk8 = W.T*32, qsT/khT evacuate to fp8 at
# 512x/32x their true scale, q_emb x32, k_emb x512; the 16384x logit scale
# is removed for free inside the Exp activation (scale=1/16384).
#
# PSUM discipline: each 2KB PSUM bank holds exactly one accumulation group;
# the first matmul of a group has start=True (hardware zeroes the whole
# bank), every other matmul accumulates (start=False), including ones that
# touch bytes no earlier instruction wrote (they add onto the zeroed bank).
#
# Heads are packed even/odd into the two 64-partition halves so the per-head
# 64x64 logits matmuls and the 64-row attention matmuls run as concurrent
# PE row/column tiles (tile_position diagonal packing).  Large PSUM->SBUF
# evacuations are spread across the Scalar/Vector/GpSimd engines.

import numpy as np

H = 8          # heads
QK = 64        # per-head qk/vo channels
C = 512        # io channels
X = 256        # spatial H (attention contraction axis)
W = 256        # spatial W (vmapped axis, sharded)
N_CORES = 8
WC = W // N_CORES   # w columns per core
PAIRS = WC // 2

_CACHE = {}


def _build_program():
    import concourse.mybir as mybir
    import concourse.tile as tile
    from concourse import bacc

    f32 = mybir.dt.float32
    bf16 = mybir.dt.bfloat16
    f8 = mybir.dt.float8e4
    AF = mybir.ActivationFunctionType
    DR = mybir.MatmulPerfMode.DoubleRow

    nc = bacc.Bacc("TRN2", target_bir_lowering=False, debug=False,
                   num_devices=N_CORES)

    qin = nc.dram_tensor("qin", [PAIRS, C, 2, X], f8, kind="ExternalInput").ap()
    kin = nc.dram_tensor("kin", [PAIRS, C, 2, X], f8, kind="ExternalInput").ap()
    vin = nc.dram_tensor("vin", [PAIRS, C, 2, X], bf16, kind="ExternalInput").ap()
    wq8 = nc.dram_tensor("wq8", [C, C], f8, kind="ExternalInput").ap()
    wk8 = nc.dram_tensor("wk8", [C, C], f8, kind="ExternalInput").ap()
    wvt = nc.dram_tensor("wvt", [C, C], bf16, kind="ExternalInput").ap()
    wot = nc.dram_tensor("wot", [C, C], bf16, kind="ExternalInput").ap()
    qe8 = nc.dram_tensor("qe8", [X, 2 * QK], f8, kind="ExternalInput").ap()
    ke8 = nc.dram_tensor("ke8", [X, 2 * QK], f8, kind="ExternalInput").ap()
    vet = nc.dram_tensor("vet", [QK, X], f32, kind="ExternalInput").ap()
    out = nc.dram_tensor("out", [C, WC, X], f32, kind="ExternalOutput").ap()

    KT = C // 128   # 4 contraction tiles of the channel dim
    XT = X // 128   # 2 tiles of the spatial-x dim
    ELS = 1.0 / 16384.0   # logit descale folded into Exp

    with tile.TileContext(nc) as tc:
        with (
            tc.tile_pool(name="consts", bufs=1) as consts,
            tc.tile_pool(name="inp", bufs=3) as inp,
            tc.tile_pool(name="qkt", bufs=2) as qkt,
            tc.tile_pool(name="mid", bufs=2) as mid,
            tc.tile_pool(name="small", bufs=8) as small,
            tc.tile_pool(name="psA", bufs=3, space="PSUM") as psA,
            tc.tile_pool(name="psVL", bufs=2, space="PSUM") as psVL,
            tc.tile_pool(name="psU", bufs=3, space="PSUM") as psU,
        ):
            def load_inputs(pair):
                # fp8 q/k: channel = (kp*2 + ki)*128 + p  ->  [p, kp, ki, w, x]
                q_t = inp.tile([128, 2, 2, 2, X], f8, tag="q_t")
                nc.sync.dma_start(
                    q_t[:], qin[pair].rearrange(
                        "(kp ki p) w x -> p kp ki (w x)", p=128, ki=2))
                k_t = inp.tile([128, 2, 2, 2, X], f8, tag="k_t")
                nc.sync.dma_start(
                    k_t[:], kin[pair].rearrange(
                        "(kp ki p) w x -> p kp ki (w x)", p=128, ki=2))
                v_t = inp.tile([128, KT, 2, X], bf16, tag="v_t")
                nc.sync.dma_start(
                    v_t[:], vin[pair].rearrange("(kt p) w x -> p kt (w x)", p=128))
                return q_t, k_t, v_t

            # pair-0 inputs first so the PE can start ASAP; q is split per
            # k-tile so the first matmul only waits for one chunk. Constants
            # go on the ACT HWDGE ring so the two DMA streams run in parallel.
            q0 = inp.tile([128, 2, 2, 2, X], f8, tag="q_t")
            for kp in range(2):
                nc.sync.dma_start(
                    q0[:, kp, :, :, :],
                    qin[0, kp * 256:(kp + 1) * 256].rearrange(
                        "(ki p) w x -> p ki w x", p=128))
            k0 = inp.tile([128, 2, 2, 2, X], f8, tag="k_t")
            nc.sync.dma_start(
                k0[:], kin[0].rearrange(
                    "(kp ki p) w x -> p kp ki (w x)", p=128, ki=2))
            v0 = inp.tile([128, KT, 2, X], bf16, tag="v_t")
            nc.sync.dma_start(
                v0[:], vin[0].rearrange("(kt p) w x -> p kt (w x)", p=128))
            prefetched = (q0, k0, v0)

            wq_sb = consts.tile([128, 2, 2, C], f8)
            nc.scalar.dma_start(wq_sb[:], wq8.rearrange(
                "(kp ki p) o -> p kp ki o", p=128, ki=2))
            wk_sb = consts.tile([128, 2, 2, C], f8)
            nc.scalar.dma_start(wk_sb[:], wk8.rearrange(
                "(kp ki p) o -> p kp ki o", p=128, ki=2))
            wv_sb = consts.tile([128, KT, C], bf16)
            nc.scalar.dma_start(wv_sb[:], wvt.rearrange("(kt p) o -> p kt o", p=128))
            wo_sb = consts.tile([128, KT, C], bf16)
            nc.scalar.dma_start(wo_sb[:], wot.rearrange("(kt p) o -> p kt o", p=128))
            qe_sb = consts.tile([128, XT, 2 * QK], f8)
            nc.scalar.dma_start(qe_sb[:], qe8.rearrange("(xt p) m -> p xt m", p=128))
            ke_sb = consts.tile([128, XT, 2 * QK], f8)
            nc.scalar.dma_start(ke_sb[:], ke8.rearrange("(xt p) m -> p xt m", p=128))
            ve_sb = consts.tile([128, 1, X], f32)
            nc.scalar.dma_start(ve_sb[0:QK, 0, :], vet[:])
            nc.scalar.dma_start(ve_sb[QK:128, 0, :], vet[:])


            def emit_oproj(attn, w0):
                # output projection (bf16), deferred one pair so the attn
                # evacuations never stall the PE
                for ot in range(KT):
                    po = psVL.tile([128, 2, X], f32, tag="vl")
                    for kt in range(KT):
                        nc.tensor.matmul(
                            po[:],
                            wo_sb[:, kt, ot * 128:(ot + 1) * 128],
                            attn[:, kt, :, :],
                            start=(kt == 0), stop=(kt == KT - 1))
                    ob = mid.tile([128, 2, X], f32, tag="ob")
                    nc.vector.tensor_copy(ob[:], po[:])
                    nc.sync.dma_start(
                        out[ot * 128:(ot + 1) * 128, w0:w0 + 2, :], ob[:])

            prev = None
            for pair in range(PAIRS):
                w0 = pair * 2
                q_t, k_t, v_t = prefetched if pair == 0 else load_inputs(pair)

                # --- q/k projections (fp8 DoubleRow), transposed layout:
                #     qsT/khT [x, (h c)] evacuated straight to fp8 ---
                qsT = qkt.tile([128, 2, XT, C], f8)   # [x_p, w, xt, o]
                khT = qkt.tile([128, 2, XT, C], f8)
                for wi in range(2):
                    for xt in range(XT):
                        xs = slice(xt * 128, (xt + 1) * 128)
                        pq = psA.tile([128, C], f32, tag="mm")
                        first = True
                        for kp in range(2):
                            for nh in range(2):
                                ns = slice(nh * 256, (nh + 1) * 256)
                                nc.tensor.matmul(
                                    pq[:, ns],
                                    q_t[:, kp, :, wi, xs],
                                    wq_sb[:, kp, :, ns],
                                    start=first,
                                    stop=(kp == 1 and nh == 1),
                                    perf_mode=DR)
                                first = False
                        nc.scalar.activation(qsT[:, wi, xt, :], pq[:], AF.Copy)
                        pk = psA.tile([128, C], f32, tag="mm")
                        first = True
                        for kp in range(2):
                            for nh in range(2):
                                ns = slice(nh * 256, (nh + 1) * 256)
                                nc.tensor.matmul(
                                    pk[:, ns],
                                    k_t[:, kp, :, wi, xs],
                                    wk_sb[:, kp, :, ns],
                                    start=first,
                                    stop=(kp == 1 and nh == 1),
                                    perf_mode=DR)
                                first = False
                        nc.vector.tensor_copy(khT[:, wi, xt, :], pk[:])

                if prev is not None:
                    emit_oproj(*prev)

                # --- v projection + ve add + ones column (bf16) ---
                vplus = mid.tile([128, KT, 2, X + 2], bf16)  # [c2_p, hp, w, x+2]
                for ot in range(KT):
                    pv = psVL.tile([128, 2, X], f32, tag="vl")
                    for kt in range(KT):
                        nc.tensor.matmul(
                            pv[:],
                            wv_sb[:, kt, ot * 128:(ot + 1) * 128],
                            v_t[:, kt, :, :],
                            start=(kt == 0), stop=(kt == KT - 1))
                    nc.vector.tensor_add(
                        vplus[:, ot, :, 0:X], pv[:],
                        ve_sb[:].broadcast_to([128, 2, X]))
                nc.gpsimd.memset(vplus[:, :, :, X:X + 2], 1.0)

                # --- per-w attention: logits+exp for both w first (so the
                #     Exp latency of w0 hides behind w1's logits), then U ---
                attn = mid.tile([128, KT, 2, X], bf16)  # [(h c)_p, kt, w, x]
                e_ts = []
                for wi in range(2):
                    pl = psVL.tile([128, C], f32, tag="vl")
                    # k_emb and q_emb terms, all heads at once (dup'd tables),
                    # then the per-head kh^T @ qs terms; one accumulation
                    # group for the whole bank.
                    for nh in range(2):
                        ns = slice(nh * 256, (nh + 1) * 256)
                        nc.tensor.matmul(pl[:, ns], ke_sb[:], khT[:, wi, :, ns],
                                         start=(nh == 0), stop=False,
                                         perf_mode=DR)
                    for nh in range(2):
                        ns = slice(nh * 256, (nh + 1) * 256)
                        nc.tensor.matmul(pl[:, ns], qe_sb[:], qsT[:, wi, :, ns],
                                         start=False, stop=False,
                                         perf_mode=DR)
                    # Per-head terms, one full-width DR matmul per head PAIR
                    # (2t, 2t+1): stationary = khT cols [128t, 128t+128) maps
                    # head 2t's channels to partitions 0:64 and head 2t+1's to
                    # 64:128 — exactly the even/odd packing.  The off-diagonal
                    # quadrants get cross-head garbage, but U only ever reads
                    # e_t[0:64, even-blocks] and e_t[64:128, odd-blocks].
                    for t in range(KT):
                        ts_ = slice(t * 128, (t + 1) * 128)
                        nc.tensor.matmul(
                            pl[:, ts_],
                            khT[:, wi, :, ts_],
                            qsT[:, wi, :, ts_],
                            start=False, stop=(t == KT - 1),
                            perf_mode=DR)
                    e_t = mid.tile([128, C], bf16, tag="e")
                    nc.scalar.activation(e_t[:], pl[:], AF.Exp, scale=ELS)
                    e_ts.append(e_t)

                for wi in range(2):
                    e_t = e_ts[wi]
                    for t in range(KT):          # head pairs (2t, 2t+1)
                        pu = psU.tile([128, X + 2], f32, tag="pu")
                        for j in range(2):       # j=0 even head, j=1 odd head
                            h = 2 * t + j
                            half = j * QK
                            nc.tensor.matmul(
                                pu[half:half + QK, :],
                                e_t[half:half + QK, h * QK:(h + 1) * QK],
                                vplus[half:half + QK, t, wi, :],
                                start=True, stop=True,
                                tile_position=(half, half))
                        recip = small.tile([128, 1], f32, tag="recip")
                        nc.vector.reciprocal(recip[:], pu[:, X:X + 1])
                        nc.scalar.activation(
                            attn[:, t, wi, :],
                            pu[:, 0:X], AF.Copy, scale=recip[:])

                prev = (attn, w0)

            emit_oproj(*prev)

    nc.compile()
    return nc


def _get_program():
    if "nc" not in _CACHE:
        _CACHE["nc"] = _build_program()
    return _CACHE["nc"]


def _make_in_maps(query, key_, value, Wq, Wk, Wv, Wo, q_emb, k_emb, v_emb):
    import ml_dtypes
    bf16 = ml_dtypes.bfloat16
    f8 = ml_dtypes.float8_e4m3
    # fp8 pre-scales: wq folds the 1/sqrt(256) softmax scale (/16*512 = *32),
    # so qsT evacuates at 512x true scale and khT at 32x; the logits come out
    # 512*32 = 16384x and the Exp activation descales them for free.
    wq8v = np.ascontiguousarray((Wq.T * np.float32(32.0)).astype(f8))
    wk8v = np.ascontiguousarray((Wk.T * np.float32(32.0)).astype(f8))
    wvt = np.ascontiguousarray(Wv.T.astype(bf16))
    wot = np.ascontiguousarray(Wo.T.astype(bf16))
    qe8v = np.ascontiguousarray(
        np.concatenate([q_emb, q_emb], axis=1).astype(np.float32) * 32.0
    ).astype(f8)
    ke8v = np.ascontiguousarray(
        np.concatenate([k_emb, k_emb], axis=1).astype(np.float32) * 512.0
    ).astype(f8)
    vet = np.ascontiguousarray(v_emb.T)

    def shard8(a, ws):
        return np.ascontiguousarray(
            a[:, :, ws].reshape(C, X, PAIRS, 2).transpose(2, 0, 3, 1).astype(f8))

    def shardb(a, ws):
        return np.ascontiguousarray(
            a[:, :, ws].reshape(C, X, PAIRS, 2).transpose(2, 0, 3, 1).astype(bf16))

    in_maps = []
    for c in range(N_CORES):
        ws = slice(c * WC, (c + 1) * WC)
        in_maps.append({
            "qin": shard8(query, ws),
            "kin": shard8(key_, ws),
            "vin": shardb(value, ws),
            "wq8": wq8v, "wk8": wk8v, "wvt": wvt, "wot": wot,
            "qe8": qe8v, "ke8": ke8v, "vet": vet,
        })
    return in_maps


def _run(in_maps, trace=False):
    from concourse.bass_utils import run_bass_kernel_spmd
    nc = _get_program()
    return run_bass_kernel_spmd(nc, in_maps, list(range(N_CORES)), trace=trace)


def kernel(query, key_, value, Wq, Wk, Wv, Wo, q_emb, k_emb, v_emb):
    args = (query, key_, value, Wq, Wk, Wv, Wo, q_emb, k_emb, v_emb)
    in_maps = _make_in_maps(*[np.ascontiguousarray(a, np.float32) for a in args])
    res = _run(in_maps, trace=False)
    out = np.empty((C, X, W), np.float32)
    for c in range(N_CORES):
        out[:, :, c * WC:(c + 1) * WC] = res.results[c]["out"].transpose(0, 2, 1)
    return out


# revision 24
# speedup vs baseline: 1.2164x; 1.1042x over previous
# Trainium2 Bass kernel for nn_AxialAttention (8 NeuronCores, W-parallel).
#
# Sharding: the W axis (axis=2, the vmapped axis) is split into 8 contiguous
# slices of 32 columns, one per core. Every part of the computation (the four
# 1x1-conv GEMMs, the per-(head, w) axial attention, the embedding terms) is
# independent across w, so there are no collectives; the small weight matrices
# and embedding tables are replicated to every core.
#
# Per-core math for one w column (all heads):
#   qsT[x, (h c)] = query[:, :, w].T @ (Wq.T / 16)    (scale folded into Wq)
#   khT[x, (h c)] = key_[:, :, w].T @ Wk.T
#   vh [(h c), x] = Wv @ value[:, :, w]
#   logits_h[C, c] = khT_h.T @ qsT_h + q_emb.T @ qsT_h + k_emb.T @ khT_h
#   E = exp(logits)             (max-subtraction unnecessary: |logits| < ~2)
#   U_h = E_h.T @ [vh_h + ve | 1]          (ones column gives the softmax
#   attn_h = U_h[:, :256] / U_h[:, 256]     denominator for free)
#   out[:, :, w] = Wo @ attn
#
# Precision strategy (validated against the fp32 reference, rel-err ~1.1e-2
# vs the 2e-2 gate): the q/k projections and all three logits matmul terms
# run in fp8 E4M3 with DoubleRow perf mode (2x PE throughput); the softmax
# smooths the fp8 error.  The v/o projections stay bf16 (fp8 there is a
# direct additive error path and fails the gate).  Power-of-2 pre-scales
# keep fp8 operands in range: wq8/wk8 = W.T*32, qsT/khT evacuate to fp8 at
# 512x/32x their true scale, q_emb x32, k_emb x512; the 16384x logit scale
# is removed for free inside the Exp activation (scale=1/16384).
#
# PSUM discipline: each 2KB PSUM bank holds exactly one accumulation group;
# the first matmul of a group has start=True (hardware zeroes the whole
# bank), every other matmul accumulates (start=False), including ones that
# touch bytes no earlier instruction wrote (they add onto the zeroed bank).
#
# Heads are packed even/odd into the two 64-partition halves so the per-head
# 64x64 logits matmuls and the 64-row attention matmuls run as concurrent
# PE row/column tiles (tile_position diagonal packing).  Large PSUM->SBUF
# evacuations are spread across the Scalar/Vector/GpSimd engines.

import numpy as np

H = 8          # heads
QK = 64        # per-head qk/vo channels
C = 512        # io channels
X = 256        # spatial H (attention contraction axis)
W = 256        # spatial W (vmapped axis, sharded)
N_CORES = 8
WC = W // N_CORES   # w columns per core
PAIRS = WC // 2

_CACHE = {}


def _build_program():
    import concourse.mybir as mybir
    import concourse.tile as tile
    from concourse import bacc

    f32 = mybir.dt.float32
    bf16 = mybir.dt.bfloat16
    f8 = mybir.dt.float8e4
    AF = mybir.ActivationFunctionType
    DR = mybir.MatmulPerfMode.DoubleRow

    nc = bacc.Bacc("TRN2", target_bir_lowering=False, debug=False,
                   num_devices=N_CORES)

    qin = nc.dram_tensor("qin", [PAIRS, C, 2, X], f8, kind="ExternalInput").ap()
    kin = nc.dram_tensor("kin", [PAIRS, C, 2, X], f8, kind="ExternalInput").ap()
    vin = nc.dram_tensor("vin", [PAIRS, C, 2, X], bf16, kind="ExternalInput").ap()
    wq8 = nc.dram_tensor("wq8", [C, C], f8, kind="ExternalInput").ap()
    wk8 = nc.dram_tensor("wk8", [C, C], f8, kind="ExternalInput").ap()
    wvt = nc.dram_tensor("wvt", [C, C], bf16, kind="ExternalInput").ap()
    wot = nc.dram_tensor("wot", [C, C], bf16, kind="ExternalInput").ap()
    qe8 = nc.dram_tensor("qe8", [X, 2 * QK], f8, kind="ExternalInput").ap()
    ke8 = nc.dram_tensor("ke8", [X, 2 * QK], f8, kind="ExternalInput").ap()
    vet = nc.dram_tensor("vet", [QK, X], f32, kind="ExternalInput").ap()
    out = nc.dram_tensor("out", [C, WC, X], f32, kind="ExternalOutput").ap()

    KT = C // 128   # 4 contraction tiles of the channel dim
    XT = X // 128   # 2 tiles of the spatial-x dim
    ELS = 1.0 / 16384.0   # logit descale folded into Exp

    with tile.TileContext(nc) as tc:
        with (
            tc.tile_pool(name="consts", bufs=1) as consts,
            tc.tile_pool(name="inp", bufs=3) as inp,
            tc.tile_pool(name="qkt", bufs=2) as qkt,
            tc.tile_pool(name="mid", bufs=2) as mid,
            tc.tile_pool(name="small", bufs=8) as small,
            tc.tile_pool(name="psA", bufs=3, space="PSUM") as psA,
            tc.tile_pool(name="psVL", bufs=2, space="PSUM") as psVL,
            tc.tile_pool(name="psU", bufs=3, space="PSUM") as psU,
        ):
            def load_inputs(pair):
                # fp8 q/k: channel = (kp*2 + ki)*128 + p  ->  [p, kp, ki, w, x]
                q_t = inp.tile([128, 2, 2, 2, X], f8, tag="q_t")
                nc.sync.dma_start(
                    q_t[:], qin[pair].rearrange(
                        "(kp ki p) w x -> p kp ki (w x)", p=128, ki=2))
                k_t = inp.tile([128, 2, 2, 2, X], f8, tag="k_t")
                nc.sync.dma_start(
                    k_t[:], kin[pair].rearrange(
                        "(kp ki p) w x -> p kp ki (w x)", p=128, ki=2))
                v_t = inp.tile([128, KT, 2, X], bf16, tag="v_t")
                nc.sync.dma_start(
                    v_t[:], vin[pair].rearrange("(kt p) w x -> p kt (w x)", p=128))
                return q_t, k_t, v_t

            # pair-0 inputs first so the PE can start ASAP; q is split per
            # k-tile so the first matmul only waits for one chunk. Constants
            # go on the ACT HWDGE ring so the two DMA streams run in parallel.
            q0 = inp.tile([128, 2, 2, 2, X], f8, tag="q_t")
            for kp in range(2):
                nc.sync.dma_start(
                    q0[:, kp, :, :, :],
                    qin[0, kp * 256:(kp + 1) * 256].rearrange(
                        "(ki p) w x -> p ki w x", p=128))
            k0 = inp.tile([128, 2, 2, 2, X], f8, tag="k_t")
            nc.sync.dma_start(
                k0[:], kin[0].rearrange(
                    "(kp ki p) w x -> p kp ki (w x)", p=128, ki=2))
            v0 = inp.tile([128, KT, 2, X], bf16, tag="v_t")
            nc.sync.dma_start(
                v0[:], vin[0].rearrange("(kt p) w x -> p kt (w x)", p=128))
            prefetched = (q0, k0, v0)

            wq_sb = consts.tile([128, 2, 2, C], f8)
            nc.scalar.dma_start(wq_sb[:], wq8.rearrange(
                "(kp ki p) o -> p kp ki o", p=128, ki=2))
            wk_sb = consts.tile([128, 2, 2, C], f8)
            nc.scalar.dma_start(wk_sb[:], wk8.rearrange(
                "(kp ki p) o -> p kp ki o", p=128, ki=2))
            wv_sb = consts.tile([128, KT, C], bf16)
            nc.scalar.dma_start(wv_sb[:], wvt.rearrange("(kt p) o -> p kt o", p=128))
            wo_sb = consts.tile([128, KT, C], bf16)
            nc.scalar.dma_start(wo_sb[:], wot.rearrange("(kt p) o -> p kt o", p=128))
            qe_sb = consts.tile([128, XT, 2 * QK], f8)
            nc.scalar.dma_start(qe_sb[:], qe8.rearrange("(xt p) m -> p xt m", p=128))
            ke_sb = consts.tile([128, XT, 2 * QK], f8)
            nc.scalar.dma_start(ke_sb[:], ke8.rearrange("(xt p) m -> p xt m", p=128))
            ve_sb = consts.tile([128, 1, X], f32)
            nc.scalar.dma_start(ve_sb[0:QK, 0, :], vet[:])
            nc.scalar.dma_start(ve_sb[QK:128, 0, :], vet[:])


            def emit_oproj(attn, w0):
                # output projection (bf16), deferred one pair so the attn
                # evacuations never stall the PE
                for ot in range(KT):
                    po = psVL.tile([128, 2, X], f32, tag="vl")
                    for kt in range(KT):
                        nc.tensor.matmul(
                            po[:],
                            wo_sb[:, kt, ot * 128:(ot + 1) * 128],
                            attn[:, kt, :, :],
                            start=(kt == 0), stop=(kt == KT - 1))
                    ob = mid.tile([128, 2, X], f32, tag="ob")
                    nc.vector.tensor_copy(ob[:], po[:])
                    nc.sync.dma_start(
                        out[ot * 128:(ot + 1) * 128, w0:w0 + 2, :], ob[:])

            prev = None
            for pair in range(PAIRS):
                w0 = pair * 2
                q_t, k_t, v_t = prefetched if pair == 0 else load_inputs(pair)

                # --- q/k projections (fp8 DoubleRow), transposed layout:
                #     qsT/khT [x, (h c)] evacuated straight to fp8 ---
                qsT = qkt.tile([128, 2, XT, C], f8)   # [x_p, w, xt, o]
                khT = qkt.tile([128, 2, XT, C], f8)
                for wi in range(2):
                    for xt in range(XT):
                        xs = slice(xt * 128, (xt + 1) * 128)
                        pq = psA.tile([128, C], f32, tag="mm")
                        first = True
                        for kp in range(2):
                            for nh in range(2):
                                ns = slice(nh * 256, (nh + 1) * 256)
                                nc.tensor.matmul(
                                    pq[:, ns],
                                    q_t[:, kp, :, wi, xs],
                                    wq_sb[:, kp, :, ns],
                                    start=first,
                                    stop=(kp == 1 and nh == 1),
                                    perf_mode=DR)
                                first = False
                        nc.scalar.activation(qsT[:, wi, xt, :], pq[:], AF.Copy)
                        pk = psA.tile([128, C], f32, tag="mm")
                        first = True
                        for kp in range(2):
                            for nh in range(2):
                                ns = slice(nh * 256, (nh + 1) * 256)
                                nc.tensor.matmul(
                                    pk[:, ns],
                                    k_t[:, kp, :, wi, xs],
                                    wk_sb[:, kp, :, ns],
                                    start=first,
                                    stop=(kp == 1 and nh == 1),
                                    perf_mode=DR)
                                first = False
                        nc.vector.tensor_copy(khT[:, wi, xt, :], pk[:])

                # --- v projection + ve add + ones column (bf16) ---
                vplus = mid.tile([128, KT, 2, X + 2], bf16)  # [c2_p, hp, w, x+2]
                for ot in range(KT):
                    pv = psVL.tile([128, 2, X], f32, tag="vl")
                    for kt in range(KT):
                        nc.tensor.matmul(
                            pv[:],
                            wv_sb[:, kt, ot * 128:(ot + 1) * 128],
                            v_t[:, kt, :, :],
                            start=(kt == 0), stop=(kt == KT - 1))
                    nc.vector.tensor_add(
                        vplus[:, ot, :, 0:X], pv[:],
                        ve_sb[:].broadcast_to([128, 2, X]))
                nc.gpsimd.memset(vplus[:, :, :, X:X + 2], 1.0)

                # --- per-w attention: logits+exp for both w first (so the
                #     Exp latency of w0 hides behind w1's logits), then U ---
                attn = mid.tile([128, KT, 2, X], bf16)  # [(h c)_p, kt, w, x]
                e_ts = []
                for wi in range(2):
                    pl = psVL.tile([128, C], f32, tag="vl")
                    # k_emb and q_emb terms, all heads at once (dup'd tables),
                    # then the per-head kh^T @ qs terms; one accumulation
                    # group for the whole bank.
                    for nh in range(2):
                        ns = slice(nh * 256, (nh + 1) * 256)
                        nc.tensor.matmul(pl[:, ns], ke_sb[:], khT[:, wi, :, ns],
                                         start=(nh == 0), stop=False,
                                         perf_mode=DR)
                    for nh in range(2):
                        ns = slice(nh * 256, (nh + 1) * 256)
                        nc.tensor.matmul(pl[:, ns], qe_sb[:], qsT[:, wi, :, ns],
                                         start=False, stop=False,
                                         perf_mode=DR)
                    # Per-head terms, one full-width DR matmul per head PAIR
                    # (2t, 2t+1): stationary = khT cols [128t, 128t+128) maps
                    # head 2t's channels to partitions 0:64 and head 2t+1's to
                    # 64:128 — exactly the even/odd packing.  The off-diagonal
                    # quadrants get cross-head garbage, but U only ever reads
                    # e_t[0:64, even-blocks] and e_t[64:128, odd-blocks].
                    for t in range(KT):
                        ts_ = slice(t * 128, (t + 1) * 128)
                        nc.tensor.matmul(
                            pl[:, ts_],
                            khT[:, wi, :, ts_],
                            qsT[:, wi, :, ts_],
                            start=False, stop=(t == KT - 1),
                            perf_mode=DR)
                    e_t = mid.tile([128, C], bf16, tag="e")
                    nc.scalar.activation(e_t[:], pl[:], AF.Exp, scale=ELS)
                    e_ts.append(e_t)

                for wi in range(2):
                    e_t = e_ts[wi]
                    if wi == 1 and prev is not None:
                        # o-proj of the previous pair lands here: its PE work
                        # covers the pu-bank evacuation latency of U(w0)
                        emit_oproj(*prev)
                    for t in range(KT):          # head pairs (2t, 2t+1)
                        # the 4th chain borrows a psVL bank (free at this
                        # point) so U never waits on a pu-bank evacuation
                        pool = psU if t < KT - 1 else psVL
                        pu = pool.tile([128, X + 2], f32,
                                       tag="pu" if t < KT - 1 else "vl")
                        for j in range(2):       # j=0 even head, j=1 odd head
                            h = 2 * t + j
                            half = j * QK
                            nc.tensor.matmul(
                                pu[half:half + QK, :],
                                e_t[half:half + QK, h * QK:(h + 1) * QK],
                                vplus[half:half + QK, t, wi, :],
                                start=True, stop=True,
                                tile_position=(half, half))
                        recip = small.tile([128, 1], f32, tag="recip")
                        nc.vector.reciprocal(recip[:], pu[:, X:X + 1])
                        nc.scalar.activation(
                            attn[:, t, wi, :],
                            pu[:, 0:X], AF.Copy, scale=recip[:])

                prev = (attn, w0)

            emit_oproj(*prev)

    nc.compile()
    return nc


def _get_program():
    if "nc" not in _CACHE:
        _CACHE["nc"] = _build_program()
    return _CACHE["nc"]


def _make_in_maps(query, key_, value, Wq, Wk, Wv, Wo, q_emb, k_emb, v_emb):
    import ml_dtypes
    bf16 = ml_dtypes.bfloat16
    f8 = ml_dtypes.float8_e4m3
    # fp8 pre-scales: wq folds the 1/sqrt(256) softmax scale (/16*512 = *32),
    # so qsT evacuates at 512x true scale and khT at 32x; the logits come out
    # 512*32 = 16384x and the Exp activation descales them for free.
    wq8v = np.ascontiguousarray((Wq.T * np.float32(32.0)).astype(f8))
    wk8v = np.ascontiguousarray((Wk.T * np.float32(32.0)).astype(f8))
    wvt = np.ascontiguousarray(Wv.T.astype(bf16))
    wot = np.ascontiguousarray(Wo.T.astype(bf16))
    qe8v = np.ascontiguousarray(
        np.concatenate([q_emb, q_emb], axis=1).astype(np.float32) * 32.0
    ).astype(f8)
    ke8v = np.ascontiguousarray(
        np.concatenate([k_emb, k_emb], axis=1).astype(np.float32) * 512.0
    ).astype(f8)
    vet = np.ascontiguousarray(v_emb.T)

    def shard8(a, ws):
        return np.ascontiguousarray(
            a[:, :, ws].reshape(C, X, PAIRS, 2).transpose(2, 0, 3, 1).astype(f8))

    def shardb(a, ws):
        return np.ascontiguousarray(
            a[:, :, ws].reshape(C, X, PAIRS, 2).transpose(2, 0, 3, 1).astype(bf16))

    in_maps = []
    for c in range(N_CORES):
        ws = slice(c * WC, (c + 1) * WC)
        in_maps.append({
            "qin": shard8(query, ws),
            "kin": shard8(key_, ws),
            "vin": shardb(value, ws),
            "wq8": wq8v, "wk8": wk8v, "wvt": wvt, "wot": wot,
            "qe8": qe8v, "ke8": ke8v, "vet": vet,
        })
    return in_maps


def _run(in_maps, trace=False):
    from concourse.bass_utils import run_bass_kernel_spmd
    nc = _get_program()
    return run_bass_kernel_spmd(nc, in_maps, list(range(N_CORES)), trace=trace)


def kernel(query, key_, value, Wq, Wk, Wv, Wo, q_emb, k_emb, v_emb):
    args = (query, key_, value, Wq, Wk, Wv, Wo, q_emb, k_emb, v_emb)
    in_maps = _make_in_maps(*[np.ascontiguousarray(a, np.float32) for a in args])
    res = _run(in_maps, trace=False)
    out = np.empty((C, X, W), np.float32)
    for c in range(N_CORES):
        out[:, :, c * WC:(c + 1) * WC] = res.results[c]["out"].transpose(0, 2, 1)
    return out


# revision 27
# speedup vs baseline: 1.2234x; 1.0058x over previous
# Trainium2 Bass kernel for nn_AxialAttention (8 NeuronCores, W-parallel).
#
# Sharding: the W axis (axis=2, the vmapped axis) is split into 8 contiguous
# slices of 32 columns, one per core. Every part of the computation (the four
# 1x1-conv GEMMs, the per-(head, w) axial attention, the embedding terms) is
# independent across w, so there are no collectives; the small weight matrices
# and embedding tables are replicated to every core.
#
# Per-core math for one w column (all heads):
#   qsT[x, (h c)] = query[:, :, w].T @ (Wq.T / 16)    (scale folded into Wq)
#   khT[x, (h c)] = key_[:, :, w].T @ Wk.T
#   vh [(h c), x] = Wv @ value[:, :, w]
#   logits_h[C, c] = khT_h.T @ qsT_h + q_emb.T @ qsT_h + k_emb.T @ khT_h
#   E = exp(logits)             (max-subtraction unnecessary: |logits| < ~2)
#   U_h = E_h.T @ [vh_h + ve | 1]          (ones column gives the softmax
#   attn_h = U_h[:, :256] / U_h[:, 256]     denominator for free)
#   out[:, :, w] = Wo @ attn
#
# Precision strategy (validated against the fp32 reference, rel-err ~1.0e-2
# vs the 2e-2 gate): the q/k projections and all three logits matmul terms
# run in fp8 E4M3 with DoubleRow perf mode (2x PE throughput); the softmax
# smooths the fp8 error.  The v/o projections stay bf16 (fp8 there is a
# direct additive error path and fails the gate).  Power-of-2 pre-scales
# keep fp8 operands in range: wq8/wk8 = W.T*32, qsT/khT evacuate to fp8 at
# 512x/32x their true scale, q_emb x32, k_emb x512; the 16384x logit scale
# is removed for free inside the Exp activation (scale=1/16384).
#
# PSUM discipline: each 2KB PSUM bank holds exactly one accumulation group;
# the first matmul of a group has start=True (hardware zeroes the whole
# bank), every other matmul accumulates (start=False), including ones that
# touch bytes no earlier instruction wrote (they add onto the zeroed bank).
#
# Per-head logits run as ONE full-width DoubleRow matmul per head PAIR
# (2t, 2t+1): the stationary khT columns [128t, 128t+128) map head 2t's
# channels to PSUM partitions 0:64 and head 2t+1's to 64:128 (the even/odd
# head packing); off-diagonal quadrants receive cross-head garbage that the
# U matmuls never read.  (DoubleRow is rejected by the ISA at PE column
# offsets != 0, so this also sidesteps tile_position entirely.)
#
# W columns are processed in groups of 4 (GW) to halve phase-switch
# overhead on the PE; the output projection of the previous group is
# emitted between attention phases so attn evacuations never gate the PE.

import numpy as np

H = 8          # heads
QK = 64        # per-head qk/vo channels
C = 512        # io channels
X = 256        # spatial H (attention contraction axis)
W = 256        # spatial W (vmapped axis, sharded)
N_CORES = 8
WC = W // N_CORES   # w columns per core
GW = 4              # w columns per inner group
GROUPS = WC // GW

_CACHE = {}


def _build_program():
    import concourse.mybir as mybir
    import concourse.tile as tile
    from concourse import bacc

    f32 = mybir.dt.float32
    bf16 = mybir.dt.bfloat16
    f8 = mybir.dt.float8e4
    AF = mybir.ActivationFunctionType
    DR = mybir.MatmulPerfMode.DoubleRow

    nc = bacc.Bacc("TRN2", target_bir_lowering=False, debug=False,
                   num_devices=N_CORES)

    qin = nc.dram_tensor("qin", [GROUPS, C, GW, X], f8, kind="ExternalInput").ap()
    kin = nc.dram_tensor("kin", [GROUPS, C, GW, X], f8, kind="ExternalInput").ap()
    vin = nc.dram_tensor("vin", [GROUPS, C, GW, X], bf16, kind="ExternalInput").ap()
    wq8 = nc.dram_tensor("wq8", [C, C], f8, kind="ExternalInput").ap()
    wk8 = nc.dram_tensor("wk8", [C, C], f8, kind="ExternalInput").ap()
    wvt = nc.dram_tensor("wvt", [C, C], bf16, kind="ExternalInput").ap()
    wot = nc.dram_tensor("wot", [C, C], bf16, kind="ExternalInput").ap()
    qe8 = nc.dram_tensor("qe8", [X, 2 * QK], f8, kind="ExternalInput").ap()
    ke8 = nc.dram_tensor("ke8", [X, 2 * QK], f8, kind="ExternalInput").ap()
    vet = nc.dram_tensor("vet", [QK, X], f32, kind="ExternalInput").ap()
    out = nc.dram_tensor("out", [C, WC, X], f32, kind="ExternalOutput").ap()

    KT = C // 128   # 4 contraction tiles of the channel dim
    XT = X // 128   # 2 tiles of the spatial-x dim
    ELS = 1.0 / 16384.0   # logit descale folded into Exp

    with tile.TileContext(nc) as tc:
        with (
            tc.tile_pool(name="consts", bufs=1) as consts,
            tc.tile_pool(name="inp", bufs=3) as inp,
            tc.tile_pool(name="qkt", bufs=2) as qkt,
            tc.tile_pool(name="mid", bufs=2) as mid,
            tc.tile_pool(name="ep", bufs=GW + 1) as ep,
            tc.tile_pool(name="small", bufs=8) as small,
            tc.tile_pool(name="psA", bufs=3, space="PSUM") as psA,
            tc.tile_pool(name="psVL", bufs=2, space="PSUM") as psVL,
            tc.tile_pool(name="psU", bufs=3, space="PSUM") as psU,
        ):
            def load_inputs(g):
                # fp8 q/k: channel = (kp*2 + ki)*128 + p  ->  [p, kp, ki, w, x]
                q_t = inp.tile([128, 2, 2, GW, X], f8, tag="q_t")
                nc.sync.dma_start(
                    q_t[:], qin[g].rearrange(
                        "(kp ki p) w x -> p kp ki (w x)", p=128, ki=2))
                k_t = inp.tile([128, 2, 2, GW, X], f8, tag="k_t")
                nc.sync.dma_start(
                    k_t[:], kin[g].rearrange(
                        "(kp ki p) w x -> p kp ki (w x)", p=128, ki=2))
                v_t = inp.tile([128, KT, GW, X], bf16, tag="v_t")
                nc.sync.dma_start(
                    v_t[:], vin[g].rearrange("(kt p) w x -> p kt (w x)", p=128))
                return q_t, k_t, v_t

            # group-0 inputs first so the PE can start ASAP; constants go on
            # the ACT HWDGE ring so the two DMA streams run in parallel.
            prefetched = load_inputs(0)

            wq_sb = consts.tile([128, 2, 2, C], f8)
            nc.scalar.dma_start(wq_sb[:], wq8.rearrange(
                "(kp ki p) o -> p kp ki o", p=128, ki=2))
            wk_sb = consts.tile([128, 2, 2, C], f8)
            nc.scalar.dma_start(wk_sb[:], wk8.rearrange(
                "(kp ki p) o -> p kp ki o", p=128, ki=2))
            wv_sb = consts.tile([128, KT, C], bf16)
            nc.scalar.dma_start(wv_sb[:], wvt.rearrange("(kt p) o -> p kt o", p=128))
            wo_sb = consts.tile([128, KT, C], bf16)
            nc.scalar.dma_start(wo_sb[:], wot.rearrange("(kt p) o -> p kt o", p=128))
            qe_sb = consts.tile([128, XT, 2 * QK], f8)
            nc.scalar.dma_start(qe_sb[:], qe8.rearrange("(xt p) m -> p xt m", p=128))
            ke_sb = consts.tile([128, XT, 2 * QK], f8)
            nc.scalar.dma_start(ke_sb[:], ke8.rearrange("(xt p) m -> p xt m", p=128))
            ve_sb = consts.tile([128, 1, X], f32)
            nc.scalar.dma_start(ve_sb[0:QK, 0, :], vet[:])
            nc.scalar.dma_start(ve_sb[QK:128, 0, :], vet[:])

            def emit_oproj(attn, w0, wh):
                # output projection (bf16) for w columns [w0+2*wh, w0+2*wh+2),
                # deferred one group so attn evacuations never stall the PE
                for ot in range(KT):
                    po = psVL.tile([128, 2, X], f32, tag="vl")
                    for kt in range(KT):
                        nc.tensor.matmul(
                            po[:],
                            wo_sb[:, kt, ot * 128:(ot + 1) * 128],
                            attn[:, kt, 2 * wh:2 * wh + 2, :],
                            start=(kt == 0), stop=(kt == KT - 1))
                    ob = mid.tile([128, 2, X], f32, tag="ob")
                    nc.vector.tensor_copy(ob[:], po[:])
                    nc.sync.dma_start(
                        out[ot * 128:(ot + 1) * 128,
                            w0 + 2 * wh:w0 + 2 * wh + 2, :], ob[:])

            prev = None
            for g in range(GROUPS):
                w0 = g * GW
                q_t, k_t, v_t = prefetched if g == 0 else load_inputs(g)

                # --- q/k projections (fp8 DoubleRow), transposed layout:
                #     qsT/khT [x, (h c)] evacuated straight to fp8 ---
                qsT = qkt.tile([128, GW, XT, C], f8)   # [x_p, w, xt, o]
                khT = qkt.tile([128, GW, XT, C], f8)
                for wi in range(GW):
                    for xt in range(XT):
                        xs = slice(xt * 128, (xt + 1) * 128)
                        pq = psA.tile([128, C], f32, tag="mm")
                        first = True
                        for kp in range(2):
                            for nh in range(2):
                                ns = slice(nh * 256, (nh + 1) * 256)
                                nc.tensor.matmul(
                                    pq[:, ns],
                                    q_t[:, kp, :, wi, xs],
                                    wq_sb[:, kp, :, ns],
                                    start=first,
                                    stop=(kp == 1 and nh == 1),
                                    perf_mode=DR)
                                first = False
                        nc.scalar.activation(qsT[:, wi, xt, :], pq[:], AF.Copy)
                        pk = psA.tile([128, C], f32, tag="mm")
                        first = True
                        for kp in range(2):
                            for nh in range(2):
                                ns = slice(nh * 256, (nh + 1) * 256)
                                nc.tensor.matmul(
                                    pk[:, ns],
                                    k_t[:, kp, :, wi, xs],
                                    wk_sb[:, kp, :, ns],
                                    start=first,
                                    stop=(kp == 1 and nh == 1),
                                    perf_mode=DR)
                                first = False
                        nc.vector.tensor_copy(khT[:, wi, xt, :], pk[:])

                # --- v projection + ve add + ones column (bf16) ---
                vplus = mid.tile([128, KT, GW, X + 2], bf16)  # [c2_p, hp, w, x+2]
                for ot in range(KT):
                    for wh in range(GW // 2):
                        pv = psVL.tile([128, 2, X], f32, tag="vl")
                        for kt in range(KT):
                            nc.tensor.matmul(
                                pv[:],
                                wv_sb[:, kt, ot * 128:(ot + 1) * 128],
                                v_t[:, kt, 2 * wh:2 * wh + 2, :],
                                start=(kt == 0), stop=(kt == KT - 1))
                        nc.vector.tensor_add(
                            vplus[:, ot, 2 * wh:2 * wh + 2, 0:X], pv[:],
                            ve_sb[:].broadcast_to([128, 2, X]))
                nc.gpsimd.memset(vplus[:, :, :, X:X + 2], 1.0)

                # --- per-w attention: logits+exp for all w first (the Exp
                #     latency hides behind the next w's logits), then U ---
                attn = mid.tile([128, KT, GW, X], bf16)  # [(h c)_p, kt, w, x]
                e_ts = []
                for wi in range(GW):
                    pl = psA.tile([128, C], f32, tag="mm")
                    # k_emb and q_emb terms, all heads at once (dup'd tables),
                    # then one merged DR matmul per head pair; a single
                    # accumulation group spans the whole bank.
                    for nh in range(2):
                        ns = slice(nh * 256, (nh + 1) * 256)
                        nc.tensor.matmul(pl[:, ns], ke_sb[:], khT[:, wi, :, ns],
                                         start=(nh == 0), stop=False,
                                         perf_mode=DR)
                    for nh in range(2):
                        ns = slice(nh * 256, (nh + 1) * 256)
                        nc.tensor.matmul(pl[:, ns], qe_sb[:], qsT[:, wi, :, ns],
                                         start=False, stop=False,
                                         perf_mode=DR)
                    for t in range(KT):
                        ts_ = slice(t * 128, (t + 1) * 128)
                        nc.tensor.matmul(
                            pl[:, ts_],
                            khT[:, wi, :, ts_],
                            qsT[:, wi, :, ts_],
                            start=False, stop=(t == KT - 1),
                            perf_mode=DR)
                    e_t = ep.tile([128, C], bf16, tag="e")
                    nc.scalar.activation(e_t[:], pl[:], AF.Exp, scale=ELS)
                    e_ts.append(e_t)

                for wi in range(GW):
                    e_t = e_ts[wi]
                    if prev is not None and wi in (1, 3):
                        # o-proj of the previous group lands between U phases:
                        # its PE work covers the pu-bank evacuation latency
                        emit_oproj(prev[0], prev[1], wi // 2)
                    for t in range(KT):          # head pairs (2t, 2t+1)
                        # the 4th chain borrows a psVL bank (free here) so U
                        # never waits on a pu-bank evacuation
                        pool = psU if t < KT - 1 else psVL
                        pu = pool.tile([128, X + 2], f32,
                                       tag="pu" if t < KT - 1 else "vl")
                        for j in range(2):       # j=0 even head, j=1 odd head
                            h = 2 * t + j
                            half = j * QK
                            nc.tensor.matmul(
                                pu[half:half + QK, :],
                                e_t[half:half + QK, h * QK:(h + 1) * QK],
                                vplus[half:half + QK, t, wi, :],
                                start=True, stop=True,
                                tile_position=(half, half))
                        recip = small.tile([128, 1], f32, tag="recip")
                        nc.vector.reciprocal(recip[:], pu[:, X:X + 1])
                        nc.scalar.activation(
                            attn[:, t, wi, :],
                            pu[:, 0:X], AF.Copy, scale=recip[:])

                prev = (attn, w0)

            emit_oproj(prev[0], prev[1], 0)
            emit_oproj(prev[0], prev[1], 1)

    nc.compile()
    return nc


def _get_program():
    if "nc" not in _CACHE:
        _CACHE["nc"] = _build_program()
    return _CACHE["nc"]


def _make_in_maps(query, key_, value, Wq, Wk, Wv, Wo, q_emb, k_emb, v_emb):
    import ml_dtypes
    bf16 = ml_dtypes.bfloat16
    f8 = ml_dtypes.float8_e4m3
    # fp8 pre-scales: wq folds the 1/sqrt(256) softmax scale (/16*512 = *32),
    # so qsT evacuates at 512x true scale and khT at 32x; the logits come out
    # 512*32 = 16384x and the Exp activation descales them for free.
    wq8v = np.ascontiguousarray((Wq.T * np.float32(32.0)).astype(f8))
    wk8v = np.ascontiguousarray((Wk.T * np.float32(32.0)).astype(f8))
    wvt = np.ascontiguousarray(Wv.T.astype(bf16))
    wot = np.ascontiguousarray(Wo.T.astype(bf16))
    qe8v = np.ascontiguousarray(
        np.concatenate([q_emb, q_emb], axis=1).astype(np.float32) * 32.0
    ).astype(f8)
    ke8v = np.ascontiguousarray(
        np.concatenate([k_emb, k_emb], axis=1).astype(np.float32) * 512.0
    ).astype(f8)
    vet = np.ascontiguousarray(v_emb.T)

    def shard(a, ws, dt):
        return np.ascontiguousarray(
            a[:, :, ws].reshape(C, X, GROUPS, GW).transpose(2, 0, 3, 1).astype(dt))

    in_maps = []
    for c in range(N_CORES):
        ws = slice(c * WC, (c + 1) * WC)
        in_maps.append({
            "qin": shard(query, ws, f8),
            "kin": shard(key_, ws, f8),
            "vin": shard(value, ws, bf16),
            "wq8": wq8v, "wk8": wk8v, "wvt": wvt, "wot": wot,
            "qe8": qe8v, "ke8": ke8v, "vet": vet,
        })
    return in_maps


def _run(in_maps, trace=False):
    from concourse.bass_utils import run_bass_kernel_spmd
    nc = _get_program()
    return run_bass_kernel_spmd(nc, in_maps, list(range(N_CORES)), trace=trace)


def kernel(query, key_, value, Wq, Wk, Wv, Wo, q_emb, k_emb, v_emb):
    args = (query, key_, value, Wq, Wk, Wv, Wo, q_emb, k_emb, v_emb)
    in_maps = _make_in_maps(*[np.ascontiguousarray(a, np.float32) for a in args])
    res = _run(in_maps, trace=False)
    out = np.empty((C, X, W), np.float32)
    for c in range(N_CORES):
        out[:, :, c * WC:(c + 1) * WC] = res.results[c]["out"].transpose(0, 2, 1)
    return out


# revision 28
# speedup vs baseline: 1.2500x; 1.0217x over previous
# Trainium2 Bass kernel for nn_AxialAttention (8 NeuronCores, W-parallel).
#
# Sharding: the W axis (axis=2, the vmapped axis) is split into 8 contiguous
# slices of 32 columns, one per core. Every part of the computation (the four
# 1x1-conv GEMMs, the per-(head, w) axial attention, the embedding terms) is
# independent across w, so there are no collectives; the small weight matrices
# and embedding tables are replicated to every core.
#
# Per-core math for one w column (all heads):
#   qsT[x, (h c)] = query[:, :, w].T @ (Wq.T / 16)    (scale folded into Wq)
#   khT[x, (h c)] = key_[:, :, w].T @ Wk.T
#   vh [(h c), x] = Wv @ value[:, :, w]
#   logits_h[C, c] = khT_h.T @ qsT_h + q_emb.T @ qsT_h + k_emb.T @ khT_h
#   E = exp(logits)             (max-subtraction unnecessary: |logits| < ~2)
#   U_h = E_h.T @ [vh_h + ve | 1]          (ones column gives the softmax
#   attn_h = U_h[:, :256] / U_h[:, 256]     denominator for free)
#   out[:, :, w] = Wo @ attn
#
# Precision strategy (validated against the fp32 reference, rel-err ~1.0e-2
# vs the 2e-2 gate): the q/k projections and all three logits matmul terms
# run in fp8 E4M3 with DoubleRow perf mode (2x PE throughput); the softmax
# smooths the fp8 error.  The v/o projections stay bf16 (fp8 there is a
# direct additive error path and fails the gate).  Power-of-2 pre-scales
# keep fp8 operands in range: wq8/wk8 = W.T*32, qsT/khT evacuate to fp8 at
# 512x/32x their true scale, q_emb x32, k_emb x512; the 16384x logit scale
# is removed for free inside the Exp activation (scale=1/16384).
#
# PSUM discipline: each 2KB PSUM bank holds exactly one accumulation group;
# the first matmul of a group has start=True (hardware zeroes the whole
# bank), every other matmul accumulates (start=False), including ones that
# touch bytes no earlier instruction wrote (they add onto the zeroed bank).
#
# Per-head logits run as ONE full-width DoubleRow matmul per head PAIR
# (2t, 2t+1): the stationary khT columns [128t, 128t+128) map head 2t's
# channels to PSUM partitions 0:64 and head 2t+1's to 64:128 (the even/odd
# head packing); off-diagonal quadrants receive cross-head garbage that the
# U matmuls never read.  (DoubleRow is rejected by the ISA at PE column
# offsets != 0, so this also sidesteps tile_position entirely.)
#
# W columns are processed in groups of 4 (GW) to halve phase-switch
# overhead on the PE; the output projection of the previous group is
# emitted between attention phases so attn evacuations never gate the PE.

import numpy as np

H = 8          # heads
QK = 64        # per-head qk/vo channels
C = 512        # io channels
X = 256        # spatial H (attention contraction axis)
W = 256        # spatial W (vmapped axis, sharded)
N_CORES = 8
WC = W // N_CORES   # w columns per core
GW = 4              # w columns per inner group
GROUPS = WC // GW

_CACHE = {}


def _build_program():
    import concourse.mybir as mybir
    import concourse.tile as tile
    from concourse import bacc

    f32 = mybir.dt.float32
    bf16 = mybir.dt.bfloat16
    f8 = mybir.dt.float8e4
    AF = mybir.ActivationFunctionType
    DR = mybir.MatmulPerfMode.DoubleRow

    nc = bacc.Bacc("TRN2", target_bir_lowering=False, debug=False,
                   num_devices=N_CORES)

    qin = nc.dram_tensor("qin", [GROUPS, C, GW, X], f8, kind="ExternalInput").ap()
    kin = nc.dram_tensor("kin", [GROUPS, C, GW, X], f8, kind="ExternalInput").ap()
    vin = nc.dram_tensor("vin", [GROUPS, C, GW, X], bf16, kind="ExternalInput").ap()
    wq8 = nc.dram_tensor("wq8", [C, C], f8, kind="ExternalInput").ap()
    wk8 = nc.dram_tensor("wk8", [C, C], f8, kind="ExternalInput").ap()
    wvt = nc.dram_tensor("wvt", [C, C], bf16, kind="ExternalInput").ap()
    wot = nc.dram_tensor("wot", [C, C], bf16, kind="ExternalInput").ap()
    qe8 = nc.dram_tensor("qe8", [X, 2 * QK], f8, kind="ExternalInput").ap()
    ke8 = nc.dram_tensor("ke8", [X, 2 * QK], f8, kind="ExternalInput").ap()
    vet = nc.dram_tensor("vet", [QK, X], f32, kind="ExternalInput").ap()
    out = nc.dram_tensor("out", [C, WC, X], f32, kind="ExternalOutput").ap()

    KT = C // 128   # 4 contraction tiles of the channel dim
    XT = X // 128   # 2 tiles of the spatial-x dim
    ELS = 1.0 / 16384.0   # logit descale folded into Exp

    with tile.TileContext(nc) as tc:
        with (
            tc.tile_pool(name="consts", bufs=1) as consts,
            tc.tile_pool(name="inp", bufs=3) as inp,
            tc.tile_pool(name="qkt", bufs=2) as qkt,
            tc.tile_pool(name="mid", bufs=2) as mid,
            tc.tile_pool(name="ep", bufs=GW + 1) as ep,
            tc.tile_pool(name="small", bufs=8) as small,
            tc.tile_pool(name="psA", bufs=3, space="PSUM") as psA,
            tc.tile_pool(name="psVL", bufs=2, space="PSUM") as psVL,
            tc.tile_pool(name="psU", bufs=3, space="PSUM") as psU,
        ):
            def load_inputs(g):
                # fp8 q/k: channel = (kp*2 + ki)*128 + p  ->  [p, kp, ki, w, x]
                q_t = inp.tile([128, 2, 2, GW, X], f8, tag="q_t")
                nc.sync.dma_start(
                    q_t[:], qin[g].rearrange(
                        "(kp ki p) w x -> p kp ki (w x)", p=128, ki=2))
                k_t = inp.tile([128, 2, 2, GW, X], f8, tag="k_t")
                nc.sync.dma_start(
                    k_t[:], kin[g].rearrange(
                        "(kp ki p) w x -> p kp ki (w x)", p=128, ki=2))
                v_t = inp.tile([128, KT, GW, X], bf16, tag="v_t")
                nc.sync.dma_start(
                    v_t[:], vin[g].rearrange("(kt p) w x -> p kt (w x)", p=128))
                return q_t, k_t, v_t

            # group-0 inputs first so the PE can start ASAP; constants go on
            # the ACT HWDGE ring so the two DMA streams run in parallel.
            prefetched = load_inputs(0)

            wq_sb = consts.tile([128, 2, 2, C], f8)
            nc.scalar.dma_start(wq_sb[:], wq8.rearrange(
                "(kp ki p) o -> p kp ki o", p=128, ki=2))
            wk_sb = consts.tile([128, 2, 2, C], f8)
            nc.scalar.dma_start(wk_sb[:], wk8.rearrange(
                "(kp ki p) o -> p kp ki o", p=128, ki=2))
            wv_sb = consts.tile([128, KT, C], bf16)
            nc.scalar.dma_start(wv_sb[:], wvt.rearrange("(kt p) o -> p kt o", p=128))
            wo_sb = consts.tile([128, KT, C], bf16)
            nc.scalar.dma_start(wo_sb[:], wot.rearrange("(kt p) o -> p kt o", p=128))
            qe_sb = consts.tile([128, XT, 2 * QK], f8)
            nc.scalar.dma_start(qe_sb[:], qe8.rearrange("(xt p) m -> p xt m", p=128))
            ke_sb = consts.tile([128, XT, 2 * QK], f8)
            nc.scalar.dma_start(ke_sb[:], ke8.rearrange("(xt p) m -> p xt m", p=128))
            ve_sb = consts.tile([128, 1, X], f32)
            nc.scalar.dma_start(ve_sb[0:QK, 0, :], vet[:])
            nc.scalar.dma_start(ve_sb[QK:128, 0, :], vet[:])

            def emit_oproj(attn, w0, wh):
                # output projection (bf16) for w columns [w0+2*wh, w0+2*wh+2),
                # deferred one group so attn evacuations never stall the PE
                for ot in range(KT):
                    po = psVL.tile([128, 2, X], f32, tag="vl")
                    for kt in range(KT):
                        nc.tensor.matmul(
                            po[:],
                            wo_sb[:, kt, ot * 128:(ot + 1) * 128],
                            attn[:, kt, 2 * wh:2 * wh + 2, :],
                            start=(kt == 0), stop=(kt == KT - 1))
                    ob = mid.tile([128, 2, X], f32, tag="ob")
                    nc.vector.tensor_copy(ob[:], po[:])
                    nc.sync.dma_start(
                        out[ot * 128:(ot + 1) * 128,
                            w0 + 2 * wh:w0 + 2 * wh + 2, :], ob[:])

            prev = None
            for g in range(GROUPS):
                w0 = g * GW
                q_t, k_t, v_t = prefetched if g == 0 else load_inputs(g)

                # --- q/k projections (fp8 DoubleRow), transposed layout:
                #     qsT/khT [x, (h c)] evacuated straight to fp8 ---
                qsT = qkt.tile([128, GW, XT, C], f8)   # [x_p, w, xt, o]
                khT = qkt.tile([128, GW, XT, C], f8)
                for wi in range(GW):
                    for xt in range(XT):
                        xs = slice(xt * 128, (xt + 1) * 128)
                        pq = psA.tile([128, C], f32, tag="mm")
                        first = True
                        for kp in range(2):
                            for nh in range(2):
                                ns = slice(nh * 256, (nh + 1) * 256)
                                nc.tensor.matmul(
                                    pq[:, ns],
                                    q_t[:, kp, :, wi, xs],
                                    wq_sb[:, kp, :, ns],
                                    start=first,
                                    stop=(kp == 1 and nh == 1),
                                    perf_mode=DR)
                                first = False
                        nc.scalar.activation(qsT[:, wi, xt, :], pq[:], AF.Copy)
                        pk = psA.tile([128, C], f32, tag="mm")
                        first = True
                        for kp in range(2):
                            for nh in range(2):
                                ns = slice(nh * 256, (nh + 1) * 256)
                                nc.tensor.matmul(
                                    pk[:, ns],
                                    k_t[:, kp, :, wi, xs],
                                    wk_sb[:, kp, :, ns],
                                    start=first,
                                    stop=(kp == 1 and nh == 1),
                                    perf_mode=DR)
                                first = False
                        nc.vector.tensor_copy(khT[:, wi, xt, :], pk[:])

                # --- v projection + ve add + ones column (bf16) ---
                vplus = mid.tile([128, KT, GW, X + 2], bf16)  # [c2_p, hp, w, x+2]
                for ot in range(KT):
                    for wh in range(GW // 2):
                        pv = psVL.tile([128, 2, X], f32, tag="vl")
                        for kt in range(KT):
                            nc.tensor.matmul(
                                pv[:],
                                wv_sb[:, kt, ot * 128:(ot + 1) * 128],
                                v_t[:, kt, 2 * wh:2 * wh + 2, :],
                                start=(kt == 0), stop=(kt == KT - 1))
                        nc.vector.tensor_add(
                            vplus[:, ot, 2 * wh:2 * wh + 2, 0:X], pv[:],
                            ve_sb[:].broadcast_to([128, 2, X]))
                nc.gpsimd.memset(vplus[:, :, :, X:X + 2], 1.0)

                # --- per-w attention: logits+exp for all w first (the Exp
                #     latency hides behind the next w's logits), then U ---
                attn = mid.tile([128, KT, GW, X], bf16)  # [(h c)_p, kt, w, x]
                e_ts = []
                for wi in range(GW):
                    pl = psA.tile([128, C], f32, tag="mm")
                    # k_emb and q_emb terms, all heads at once (dup'd tables),
                    # then one merged DR matmul per head pair; a single
                    # accumulation group spans the whole bank.
                    for nh in range(2):
                        ns = slice(nh * 256, (nh + 1) * 256)
                        nc.tensor.matmul(pl[:, ns], ke_sb[:], khT[:, wi, :, ns],
                                         start=(nh == 0), stop=False,
                                         perf_mode=DR)
                    for nh in range(2):
                        ns = slice(nh * 256, (nh + 1) * 256)
                        nc.tensor.matmul(pl[:, ns], qe_sb[:], qsT[:, wi, :, ns],
                                         start=False, stop=False,
                                         perf_mode=DR)
                    for t in range(KT):
                        ts_ = slice(t * 128, (t + 1) * 128)
                        nc.tensor.matmul(
                            pl[:, ts_],
                            khT[:, wi, :, ts_],
                            qsT[:, wi, :, ts_],
                            start=False, stop=(t == KT - 1),
                            perf_mode=DR)
                    e_t = ep.tile([128, C], bf16, tag="e")
                    nc.scalar.activation(e_t[:], pl[:], AF.Exp, scale=ELS)
                    e_ts.append(e_t)

                for wi in range(GW):
                    e_t = e_ts[wi]
                    if prev is not None and wi in (1, 3):
                        # o-proj of the previous group lands between U phases:
                        # its PE work covers the pu-bank evacuation latency
                        emit_oproj(prev[0], prev[1], wi // 2)
                    for t in range(KT):          # head pairs (2t, 2t+1)
                        # the 4th chain borrows a psA bank (idle during U) so
                        # neither U nor the o-proj wait on pu evacuations
                        pool = psU if t < KT - 1 else psA
                        pu = pool.tile([128, X + 2], f32,
                                       tag="pu" if t < KT - 1 else "mm")
                        for j in range(2):       # j=0 even head, j=1 odd head
                            h = 2 * t + j
                            half = j * QK
                            nc.tensor.matmul(
                                pu[half:half + QK, :],
                                e_t[half:half + QK, h * QK:(h + 1) * QK],
                                vplus[half:half + QK, t, wi, :],
                                start=True, stop=True,
                                tile_position=(half, half))
                        recip = small.tile([128, 1], f32, tag="recip")
                        nc.vector.reciprocal(recip[:], pu[:, X:X + 1])
                        nc.scalar.activation(
                            attn[:, t, wi, :],
                            pu[:, 0:X], AF.Copy, scale=recip[:])

                prev = (attn, w0)

            emit_oproj(prev[0], prev[1], 0)
            emit_oproj(prev[0], prev[1], 1)

    nc.compile()
    return nc


def _get_program():
    if "nc" not in _CACHE:
        _CACHE["nc"] = _build_program()
    return _CACHE["nc"]


def _make_in_maps(query, key_, value, Wq, Wk, Wv, Wo, q_emb, k_emb, v_emb):
    import ml_dtypes
    bf16 = ml_dtypes.bfloat16
    f8 = ml_dtypes.float8_e4m3
    # fp8 pre-scales: wq folds the 1/sqrt(256) softmax scale (/16*512 = *32),
    # so qsT evacuates at 512x true scale and khT at 32x; the logits come out
    # 512*32 = 16384x and the Exp activation descales them for free.
    wq8v = np.ascontiguousarray((Wq.T * np.float32(32.0)).astype(f8))
    wk8v = np.ascontiguousarray((Wk.T * np.float32(32.0)).astype(f8))
    wvt = np.ascontiguousarray(Wv.T.astype(bf16))
    wot = np.ascontiguousarray(Wo.T.astype(bf16))
    qe8v = np.ascontiguousarray(
        np.concatenate([q_emb, q_emb], axis=1).astype(np.float32) * 32.0
    ).astype(f8)
    ke8v = np.ascontiguousarray(
        np.concatenate([k_emb, k_emb], axis=1).astype(np.float32) * 512.0
    ).astype(f8)
    vet = np.ascontiguousarray(v_emb.T)

    def shard(a, ws, dt):
        return np.ascontiguousarray(
            a[:, :, ws].reshape(C, X, GROUPS, GW).transpose(2, 0, 3, 1).astype(dt))

    in_maps = []
    for c in range(N_CORES):
        ws = slice(c * WC, (c + 1) * WC)
        in_maps.append({
            "qin": shard(query, ws, f8),
            "kin": shard(key_, ws, f8),
            "vin": shard(value, ws, bf16),
            "wq8": wq8v, "wk8": wk8v, "wvt": wvt, "wot": wot,
            "qe8": qe8v, "ke8": ke8v, "vet": vet,
        })
    return in_maps


def _run(in_maps, trace=False):
    from concourse.bass_utils import run_bass_kernel_spmd
    nc = _get_program()
    return run_bass_kernel_spmd(nc, in_maps, list(range(N_CORES)), trace=trace)


def kernel(query, key_, value, Wq, Wk, Wv, Wo, q_emb, k_emb, v_emb):
    args = (query, key_, value, Wq, Wk, Wv, Wo, q_emb, k_emb, v_emb)
    in_maps = _make_in_maps(*[np.ascontiguousarray(a, np.float32) for a in args])
    res = _run(in_maps, trace=False)
    out = np.empty((C, X, W), np.float32)
    for c in range(N_CORES):
        out[:, :, c * WC:(c + 1) * WC] = res.results[c]["out"].transpose(0, 2, 1)
    return out
